# revision 7
# baseline (speedup 1.0000x reference)
"""Trainium2 Bass kernel for nn_DCRNN_Layer (gnn_message_passing).

Strategy
--------
The reference graph is deterministic: node i has out-edges to (i+1..i+16) mod N.
Therefore each Chebyshev diffusion step is a banded circulant operator:

    prop_fwd(x)[d] = sum_{j=1..16} (n_out * x)[d-j]
    prop_rev(x)[d] = sum_{j=1..16} (n_in  * x)[d+j]

with n_out/n_in the per-node inverse (weighted) degrees.  Band application is
expressed as dense 128x128 {0,1} band-block matmuls on the TensorEngine (two
constant blocks per node-tile: same-tile band + neighbor-tile corner), and the
per-node scaling commutes with the feature matmuls so it is folded into
host-prescaled operands.

The gate computation is refactored "feature first" (props act on 64-wide gate
activations instead of the 128-wide state):

    Hh = Xc@W0' + Bf(Yf1) + Br(Yr1) + Bf(no*Bf(Yf2)) + Br(ni*Br(Yr2)) + bias
    W0' = W00+W10-W02-W12,  Yf1=(no*Xc)@W01, Yf2=(no*Xc)@(2*W02), ...

Sharding: 8 cores = batch(4) x node-halves(2).  Each core gets a 5248-node
extended shard (64-node halo each side + padding, circularly wrapped on the
host) so no inter-core communication is needed.  All matmul operands are fp16
(PSUM accumulation fp32): end-to-end absmax error vs the fp32 reference is
~2e-3 of scale (validated by host emulation).

kernel() is self-contained: host preprocessing (numpy) -> 8-core SPMD Bass
kernel -> host unshard.  If the edge structure ever deviates from the banded
pattern, a numpy fallback reproduces the reference exactly.
"""

import numpy as np
from contextlib import ExitStack

# Problem constants (hardcoded per contract).
B, N, DEG, IN, OUT, K = 4, 10000, 16, 64, 64, 3
C = IN + OUT          # 128
EPS = 1e-8
NCORES = 8
HALF = N // 2         # 5000 nodes per node-shard
T = 41                # node tiles per extended shard
N2 = T * 128          # 5248 extended shard nodes
LHALO = 64            # left halo (ext index of output node 0)

_prog_cache = {}


# ----------------------------------------------------------------------------
# Device program
# ----------------------------------------------------------------------------
def _build_program(n_tiles=T):
    import concourse.bass as bass
    import concourse.bacc as bacc
    import concourse.tile as tile
    from concourse import mybir

    f16 = mybir.dt.float16
    f32 = mybir.dt.float32
    Tn = n_tiles
    n2 = Tn * 128

    nc = bacc.Bacc(
        "TRN2",
        target_bir_lowering=False,
        debug=False,
        enable_asserts=False,
        num_devices=NCORES,
    )

    din = {}
    for name, shape, dt in [
        ("xht", [128, n2], f16),
        ("xhno", [128, n2], f16),
        ("xhni", [128, n2], f16),
        ("hext", [128, Tn * 64], f32),
        ("nout", [128, Tn], f32),
        ("nin", [128, Tn], f32),
        ("wzr", [128, 640], f16),
        ("wh", [128, 320], f16),
        ("bzr", [1, 128], f16),
        ("bh1", [1, 64], f16),
        ("bands", [128, 512], f16),
        ("ident", [128, 128], f16),
        ("ones1", [1, 128], f16),
    ]:
        din[name] = nc.dram_tensor(name, shape, dt, kind="ExternalInput").ap()
    d_out = nc.dram_tensor("out", [128, Tn * 64], f32, kind="ExternalOutput").ap()

    SIG = mybir.ActivationFunctionType.Sigmoid
    TANH = mybir.ActivationFunctionType.Tanh

    with tile.TileContext(nc) as tc:
        with ExitStack() as ctx:
            consts = ctx.enter_context(tc.tile_pool(name="consts", bufs=1))

            def load(name):
                t = consts.tile(din[name].shape, din[name].dtype, tag=name)
                nc.sync.dma_start(t[:], din[name])
                return t

            xht, xhno, xhni = load("xht"), load("xhno"), load("xhni")
            hext, nout, nin = load("hext"), load("nout"), load("nin")
            wzr, wh, bzr, bh1 = load("wzr"), load("wh"), load("bzr"), load("bh1")
            bands, ident, ones1 = load("bands"), load("ident"), load("ones1")

            LF = bands[:, 0:128]      # fwd same-tile band (lhsT layout)
            UF = bands[:, 128:256]    # fwd prev-tile corner
            LR = bands[:, 256:384]    # rev same-tile band
            UR = bands[:, 384:512]    # rev next-tile corner

            # persistent tensors written on-device
            xhr_p = consts.tile([128, n2], f16, tag="xhr_p")
            xhr_no = consts.tile([128, n2], f16, tag="xhr_no")
            xhr_ni = consts.tile([128, n2], f16, tag="xhr_ni")
            outst = consts.tile([128, Tn * 64], f32, tag="outst")
            # X-part (rows 0:64) of the XHR stationaries never changes
            nc.vector.tensor_copy(xhr_p[0:64, :], xht[0:64, :])
            nc.vector.tensor_copy(xhr_no[0:64, :], xhno[0:64, :])
            nc.vector.tensor_copy(xhr_ni[0:64, :], xhni[0:64, :])

            sc_pool = ctx.enter_context(tc.tile_pool(name="sc", bufs=5))
            sq_pool = ctx.enter_context(tc.tile_pool(name="sq", bufs=5))
            sch_pool = ctx.enter_context(tc.tile_pool(name="sch", bufs=5))
            sqh_pool = ctx.enter_context(tc.tile_pool(name="sqh", bufs=5))
            hhr_pool = ctx.enter_context(tc.tile_pool(name="hhr", bufs=3))
            rt_pool = ctx.enter_context(tc.tile_pool(name="rt", bufs=3))
            zsb_pool = ctx.enter_context(tc.tile_pool(name="zsb", bufs=6))
            fin_pool = ctx.enter_context(tc.tile_pool(name="fin", bufs=4))

            # PSUM: 8 banks total. "l1" tag holds the z/r and h gate
            # accumulators (each lives ~3 pipeline steps -> 6 concurrent);
            # "y" tag holds all short-lived transients (feature outputs,
            # level-2 prop outputs, the R transpose).
            py = ctx.enter_context(tc.tile_pool(name="py", bufs=2, space="PSUM"))
            pl1 = ctx.enter_context(tc.tile_pool(name="pl1", bufs=6, space="PSUM"))

            sc = [None] * Tn
            sq = [None] * Tn
            sch = [None] * Tn
            sqh = [None] * Tn
            l1 = [None] * Tn
            l1h = [None] * Tn
            zsb = [None] * Tn

            def xs(tsr, j):
                return tsr[:, j * 128:(j + 1) * 128]

            mm = nc.tensor.matmul

            for it in range(Tn + 6):
                # ---- P1: z/r feature matmuls for tile j -------------------
                if it < Tn:
                    j = it
                    y = py.tile([128, 512], f32, tag="y")
                    mm(y[:, 0:256], xs(xhno, j), wzr[:, 128:384],
                       start=True, stop=False)
                    mm(y[:, 256:512], xs(xhni, j), wzr[:, 384:640],
                       start=False, stop=True)
                    t1 = pl1.tile([128, 128], f32, tag="l1")
                    mm(t1[:], xs(xht, j), wzr[:, 0:128],
                       start=True, stop=False)
                    mm(t1[:], ones1[:, :], bzr[:, :],
                       start=False, stop=False)
                    l1[j] = t1
                    s = sc_pool.tile([128, 512], f16, tag="sc")
                    nc.vector.tensor_copy(s[:], y[:])
                    sc[j] = s

                # ---- P2: z/r level-1 band props for out-tile j ------------
                if 1 <= it <= Tn:
                    j = it - 1
                    t1 = l1[j]
                    # hh accumulators (group stays open until P3)
                    mm(t1[:], LF, sc[j][:, 0:128], start=False, stop=False)
                    mm(t1[:], LR, sc[j][:, 256:384], start=False, stop=False)
                    if j > 0:
                        mm(t1[:], UF, sc[j - 1][:, 0:128], start=False, stop=False)
                    if j < Tn - 1:
                        mm(t1[:], UR, sc[j + 1][:, 256:384], start=False, stop=False)
                    # q1 in a short-lived transient bank (own group)
                    q1 = py.tile([128, 256], f32, tag="y")
                    mms = [(q1[:, 0:128], LF, sc[j][:, 128:256]),
                           (q1[:, 128:256], LR, sc[j][:, 384:512])]
                    if j > 0:
                        mms.append((q1[:, 0:128], UF, sc[j - 1][:, 128:256]))
                    if j < Tn - 1:
                        mms.append((q1[:, 128:256], UR, sc[j + 1][:, 384:512]))
                    for i, (dst, st, mv) in enumerate(mms):
                        mm(dst, st, mv, start=(i == 0), stop=(i == len(mms) - 1))
                    q = sq_pool.tile([128, 256], f16, tag="sq")
                    nc.vector.tensor_scalar_mul(q[:, 0:128], q1[:, 0:128],
                                                nout[:, j:j + 1])
                    nc.vector.tensor_scalar_mul(q[:, 128:256], q1[:, 128:256],
                                                nin[:, j:j + 1])
                    sq[j] = q

                # ---- P3: z/r level-2 band props + gate finish -------------
                if 2 <= it <= Tn + 1:
                    j = it - 2
                    t1 = l1[j]
                    mms = [(LF, sq[j][:, 0:128])]
                    if j > 0:
                        mms.append((UF, sq[j - 1][:, 0:128]))
                    mms.append((LR, sq[j][:, 128:256]))
                    if j < Tn - 1:
                        mms.append((UR, sq[j + 1][:, 128:256]))
                    for i, (st, mv) in enumerate(mms):
                        mm(t1[:], st, mv, start=False, stop=(i == len(mms) - 1))
                    # Z gate: sigmoid straight from PSUM
                    z = zsb_pool.tile([128, 64], f32, tag="zsb")
                    nc.scalar.activation(z[:], t1[:, 0:64], SIG)
                    zsb[j] = z
                    # R gate: transpose to channel-major, then sigmoid
                    hr = hhr_pool.tile([128, 64], f16, tag="hhr")
                    nc.vector.tensor_copy(hr[:], t1[:, 64:128])
                    rtp = py.tile([128, 128], f16, tag="y")
                    nc.tensor.transpose(rtp[64:128, :], hr[:], ident[:])
                    rts = rt_pool.tile([128, 128], f16, tag="rt")
                    nc.scalar.activation(rts[64:128, :], rtp[64:128, :], SIG)
                    # XHR stationaries, H-part rows 64:128
                    nc.vector.tensor_mul(xs(xhr_p, j)[64:128, :],
                                         xs(xht, j)[64:128, :], rts[64:128, :])
                    nc.vector.tensor_mul(xs(xhr_no, j)[64:128, :],
                                         xs(xhno, j)[64:128, :], rts[64:128, :])
                    nc.vector.tensor_mul(xs(xhr_ni, j)[64:128, :],
                                         xs(xhni, j)[64:128, :], rts[64:128, :])

                # ---- P5: h-gate feature matmuls ---------------------------
                if 3 <= it <= Tn + 2:
                    j = it - 3
                    yh = py.tile([128, 256], f32, tag="y")
                    mm(yh[:, 0:128], xs(xhr_no, j), wh[:, 64:192],
                       start=True, stop=False)
                    mm(yh[:, 128:256], xs(xhr_ni, j), wh[:, 192:320],
                       start=False, stop=True)
                    th = pl1.tile([128, 128], f32, tag="l1")
                    mm(th[:, 0:64], xs(xhr_p, j), wh[:, 0:64],
                       start=True, stop=False)
                    mm(th[:, 0:64], ones1[:, :], bh1[:, :],
                       start=False, stop=False)
                    l1h[j] = th
                    s = sch_pool.tile([128, 256], f16, tag="sch")
                    nc.vector.tensor_copy(s[:], yh[:])
                    sch[j] = s

                # ---- P6: h level-1 band props -----------------------------
                if 4 <= it <= Tn + 3:
                    j = it - 4
                    th = l1h[j]
                    hh = th[:, 0:64]
                    mm(hh, LF, sch[j][:, 0:64], start=False, stop=False)
                    mm(hh, LR, sch[j][:, 128:192], start=False, stop=False)
                    if j > 0:
                        mm(hh, UF, sch[j - 1][:, 0:64], start=False, stop=False)
                    if j < Tn - 1:
                        mm(hh, UR, sch[j + 1][:, 128:192], start=False, stop=False)
                    q1 = py.tile([128, 128], f32, tag="y")
                    mms = [(q1[:, 0:64], LF, sch[j][:, 64:128]),
                           (q1[:, 64:128], LR, sch[j][:, 192:256])]
                    if j > 0:
                        mms.append((q1[:, 0:64], UF, sch[j - 1][:, 64:128]))
                    if j < Tn - 1:
                        mms.append((q1[:, 64:128], UR, sch[j + 1][:, 192:256]))
                    for i, (dst, st, mv) in enumerate(mms):
                        mm(dst, st, mv, start=(i == 0), stop=(i == len(mms) - 1))
                    q = sqh_pool.tile([128, 128], f16, tag="sqh")
                    nc.vector.tensor_scalar_mul(q[:, 0:64], q1[:, 0:64],
                                                nout[:, j:j + 1])
                    nc.vector.tensor_scalar_mul(q[:, 64:128], q1[:, 64:128],
                                                nin[:, j:j + 1])
                    sqh[j] = q

                # ---- P7: h level-2 + H_tilde + final combine --------------
                if 5 <= it <= Tn + 4:
                    j = it - 5
                    th = l1h[j]
                    mms = [(LF, sqh[j][:, 0:64])]
                    if j > 0:
                        mms.append((UF, sqh[j - 1][:, 0:64]))
                    mms.append((LR, sqh[j][:, 64:128]))
                    if j < Tn - 1:
                        mms.append((UR, sqh[j + 1][:, 64:128]))
                    for i, (st, mv) in enumerate(mms):
                        mm(th[:, 0:64], st, mv, start=False,
                           stop=(i == len(mms) - 1))
                    ht = fin_pool.tile([128, 64], f32, tag="ht")
                    nc.scalar.activation(ht[:], th[:, 0:64], TANH)
                    # out = ht + z * (h - ht)
                    d = fin_pool.tile([128, 64], f32, tag="fd")
                    nc.vector.tensor_sub(d[:], hext[:, j * 64:(j + 1) * 64], ht[:])
                    e = fin_pool.tile([128, 64], f32, tag="fe")
                    nc.vector.tensor_mul(e[:], d[:], zsb[j][:])
                    nc.vector.tensor_add(outst[:, j * 64:(j + 1) * 64], e[:], ht[:])

            nc.sync.dma_start(d_out, outst[:])

    nc.compile()
    return nc


def _get_program(n_tiles=T):
    key = n_tiles
    if key not in _prog_cache:
        _prog_cache[key] = _build_program(n_tiles)
    return _prog_cache[key]


# ----------------------------------------------------------------------------
# Host-side preparation
# ----------------------------------------------------------------------------
def _band_matrices():
    s = np.arange(128)[:, None]
    d = np.arange(128)[None, :]
    bf_l = ((d - s >= 1) & (d - s <= DEG)).astype(np.float16)
    bf_u = (s - d >= 128 - DEG).astype(np.float16)
    br_l = ((s - d >= 1) & (s - d <= DEG)).astype(np.float16)
    br_u = (d - s >= 128 - DEG).astype(np.float16)
    return np.concatenate([bf_l, bf_u, br_l, br_u], axis=1)  # [128, 512]


def _pack_weights_zr(Wz, Wr):
    def parts(W):
        w0 = W[0, 0] + W[1, 0] - W[0, 2] - W[1, 2]
        return w0, W[0, 1], 2.0 * W[0, 2], W[1, 1], 2.0 * W[1, 2]
    w0z, f1z, f2z, r1z, r2z = parts(Wz)
    w0r, f1r, f2r, r1r, r2r = parts(Wr)
    return np.concatenate(
        [w0z, w0r, f1z, f1r, f2z, f2r, r1z, r1r, r2z, r2r], axis=1
    ).astype(np.float16)  # [128, 640]


def _pack_weights_h(Wh):
    w0 = Wh[0, 0] + Wh[1, 0] - Wh[0, 2] - Wh[1, 2]
    return np.concatenate(
        [w0, Wh[0, 1], 2.0 * Wh[0, 2], Wh[1, 1], 2.0 * Wh[1, 2]], axis=1
    ).astype(np.float16)  # [128, 320]


def _is_banded_graph(row, col):
    if row.shape != (N * DEG,):
        return False
    r_exp = np.repeat(np.arange(N, dtype=np.int64), DEG)
    if not np.array_equal(row, r_exp):
        return False
    c_exp = (r_exp + np.tile(np.arange(1, DEG + 1, dtype=np.int64), N)) % N
    return np.array_equal(col, c_exp)


def _numpy_fallback(X, H, edge_weight, Wz, bz, Wr, br, Wh, bh, row, col):
    """Exact reference math on the host (only used if the graph deviates)."""
    deg_out = np.bincount(row, weights=edge_weight, minlength=N).astype(np.float32)
    deg_in = np.bincount(col, weights=edge_weight, minlength=N).astype(np.float32)
    norm_out = (1.0 / (deg_out + EPS))[row].astype(np.float32)
    norm_in = (1.0 / (deg_in + EPS))[col].astype(np.float32)

    def prop(x, src, dst, nrm):
        msg = x[:, src, :] * nrm[None, :, None]
        out = np.zeros_like(x)
        np.add.at(out, (slice(None), dst), msg)
        return out

    def dconv(Xc, W, b):
        Hh = Xc @ (W[0, 0] + W[1, 0])
        t1o = prop(Xc, row, col, norm_out)
        t1i = prop(Xc, col, row, norm_in)
        Hh = Hh + t1o @ W[0, 1] + t1i @ W[1, 1]
        for k in range(2, K):
            t1o = 2.0 * prop(t1o, row, col, norm_out) - Xc
            t1i = 2.0 * prop(t1i, col, row, norm_in) - Xc
            Hh = Hh + t1o @ W[0, k] + t1i @ W[1, k]
        return Hh + b

    XH = np.concatenate([X, H], axis=-1)
    Z = 1.0 / (1.0 + np.exp(-dconv(XH, Wz, bz)))
    R = 1.0 / (1.0 + np.exp(-dconv(XH, Wr, br)))
    XHR = np.concatenate([X, H * R], axis=-1)
    Ht = np.tanh(dconv(XHR, Wh, bh))
    return (Z * H + (1.0 - Z) * Ht).astype(np.float32)


def make_in_maps(X, H, edge_weight, Wz, bz, Wr, br, Wh, bh, row, col):
    """Build the 8 per-core input dicts (host sharding + preprocessing)."""
    deg_out = np.bincount(row, weights=edge_weight, minlength=N).astype(np.float32)
    deg_in = np.bincount(col, weights=edge_weight, minlength=N).astype(np.float32)
    n_out = (1.0 / (deg_out + EPS)).astype(np.float32)
    n_in = (1.0 / (deg_in + EPS)).astype(np.float32)

    shared = {
        "wzr": _pack_weights_zr(Wz, Wr),
        "wh": _pack_weights_h(Wh),
        "bzr": np.concatenate([bz, br])[None, :].astype(np.float16),
        "bh1": bh[None, :].astype(np.float16),
        "bands": _band_matrices(),
        "ident": np.eye(128, dtype=np.float16),
        "ones1": np.ones((1, 128), dtype=np.float16),
    }

    in_maps = []
    for core in range(NCORES):
        b, half = core // 2, core % 2
        g0 = half * HALF - LHALO
        idx = (g0 + np.arange(N2)) % N
        ext = np.concatenate([X[b], H[b]], axis=1)[idx]          # [N2, 128] f32
        no_e = n_out[idx]
        ni_e = n_in[idx]
        m = dict(shared)
        m["xht"] = np.ascontiguousarray(ext.T).astype(np.float16)
        m["xhno"] = np.ascontiguousarray((ext * no_e[:, None]).T).astype(np.float16)
        m["xhni"] = np.ascontiguousarray((ext * ni_e[:, None]).T).astype(np.float16)
        m["hext"] = np.ascontiguousarray(
            H[b][idx].reshape(T, 128, 64).transpose(1, 0, 2).reshape(128, T * 64)
        ).astype(np.float32)
        m["nout"] = np.ascontiguousarray(no_e.reshape(T, 128).T).astype(np.float32)
        m["nin"] = np.ascontiguousarray(ni_e.reshape(T, 128).T).astype(np.float32)
        in_maps.append(m)
    return in_maps


def unshard_outputs(results):
    out = np.empty((B, N, OUT), dtype=np.float32)
    for core in range(NCORES):
        b, half = core // 2, core % 2
        res = results[core]["out"]                                # [128, T*64]
        ext = res.reshape(128, T, 64).transpose(1, 0, 2).reshape(N2, 64)
        out[b, half * HALF:(half + 1) * HALF] = ext[LHALO:LHALO + HALF]
    return out


def kernel(X, H, edge_weight, Wz, bz, Wr, br, Wh, bh, edge_index):
    X = np.asarray(X, dtype=np.float32)
    H = np.asarray(H, dtype=np.float32)
    edge_weight = np.asarray(edge_weight, dtype=np.float32)
    Wz = np.asarray(Wz, dtype=np.float32)
    Wr = np.asarray(Wr, dtype=np.float32)
    Wh = np.asarray(Wh, dtype=np.float32)
    bz = np.asarray(bz, dtype=np.float32)
    br = np.asarray(br, dtype=np.float32)
    bh = np.asarray(bh, dtype=np.float32)
    ei = np.asarray(edge_index)
    row = ei[0].astype(np.int64)
    col = ei[1].astype(np.int64)

    if not _is_banded_graph(row, col):
        return _numpy_fallback(X, H, edge_weight, Wz, bz, Wr, br, Wh, bh,
                               row, col)

    from concourse import bass_utils

    nc = _get_program()
    in_maps = make_in_maps(X, H, edge_weight, Wz, bz, Wr, br, Wh, bh, row, col)
    res = bass_utils.run_bass_kernel_spmd(nc, in_maps, list(range(NCORES)))
    return unshard_outputs(res.results)


# revision 8
# speedup vs baseline: 1.0243x; 1.0243x over previous
"""Trainium2 Bass kernel for nn_DCRNN_Layer (gnn_message_passing).

Strategy
--------
The reference graph is deterministic: node i has out-edges to (i+1..i+16) mod N.
Therefore each Chebyshev diffusion step is a banded circulant operator:

    prop_fwd(x)[d] = sum_{j=1..16} (n_out * x)[d-j]
    prop_rev(x)[d] = sum_{j=1..16} (n_in  * x)[d+j]

with n_out/n_in the per-node inverse (weighted) degrees.  Band application is
expressed as dense 128x128 {0,1} band-block matmuls on the TensorEngine (two
constant blocks per node-tile: same-tile band + neighbor-tile corner), and the
per-node scaling commutes with the feature matmuls so it is folded into
host-prescaled operands.

The gate computation is refactored "feature first" (props act on 64-wide gate
activations instead of the 128-wide state):

    Hh = Xc@W0' + Bf(Yf1) + Br(Yr1) + Bf(no*Bf(Yf2)) + Br(ni*Br(Yr2)) + bias
    W0' = W00+W10-W02-W12,  Yf1=(no*Xc)@W01, Yf2=(no*Xc)@(2*W02), ...

Sharding: 8 cores = batch(4) x node-halves(2).  Each core gets a 5248-node
extended shard (64-node halo each side + padding, circularly wrapped on the
host) so no inter-core communication is needed.  All matmul operands are fp16
(PSUM accumulation fp32): end-to-end absmax error vs the fp32 reference is
~2e-3 of scale (validated by host emulation).

kernel() is self-contained: host preprocessing (numpy) -> 8-core SPMD Bass
kernel -> host unshard.  If the edge structure ever deviates from the banded
pattern, a numpy fallback reproduces the reference exactly.
"""

import numpy as np
from contextlib import ExitStack

# Problem constants (hardcoded per contract).
B, N, DEG, IN, OUT, K = 4, 10000, 16, 64, 64, 3
C = IN + OUT          # 128
EPS = 1e-8
NCORES = 8
HALF = N // 2         # 5000 nodes per node-shard
T = 41                # node tiles per extended shard
N2 = T * 128          # 5248 extended shard nodes
LHALO = 64            # left halo (ext index of output node 0)

_prog_cache = {}


# ----------------------------------------------------------------------------
# Device program
# ----------------------------------------------------------------------------
def _build_program(n_tiles=T, has_bias=False):
    import concourse.bass as bass
    import concourse.bacc as bacc
    import concourse.tile as tile
    from concourse import mybir

    f16 = mybir.dt.float16
    f32 = mybir.dt.float32
    Tn = n_tiles
    n2 = Tn * 128

    nc = bacc.Bacc(
        "TRN2",
        target_bir_lowering=False,
        debug=False,
        enable_asserts=False,
        num_devices=NCORES,
    )

    din = {}
    for name, shape, dt in [
        ("xht", [128, n2], f16),
        ("xhno", [128, n2], f16),
        ("xhni", [128, n2], f16),
        ("hext", [128, Tn * 64], f32),
        ("nout", [128, Tn], f32),
        ("nin", [128, Tn], f32),
        ("wzr", [128, 640], f16),
        ("wh", [128, 320], f16),
        ("bzr", [1, 128], f16),
        ("bh1", [1, 64], f16),
        ("bands", [128, 512], f16),
        ("ident", [128, 128], f16),
        ("ones1", [1, 128], f16),
    ]:
        din[name] = nc.dram_tensor(name, shape, dt, kind="ExternalInput").ap()
    d_out = nc.dram_tensor("out", [128, Tn * 64], f32, kind="ExternalOutput").ap()

    SIG = mybir.ActivationFunctionType.Sigmoid
    TANH = mybir.ActivationFunctionType.Tanh

    with tile.TileContext(nc) as tc:
        with ExitStack() as ctx:
            consts = ctx.enter_context(tc.tile_pool(name="consts", bufs=1))

            def load(name):
                t = consts.tile(din[name].shape, din[name].dtype, tag=name)
                nc.sync.dma_start(t[:], din[name])
                return t

            xht, xhno, xhni = load("xht"), load("xhno"), load("xhni")
            hext, nout, nin = load("hext"), load("nout"), load("nin")
            wzr, wh, bzr, bh1 = load("wzr"), load("wh"), load("bzr"), load("bh1")
            bands, ident, ones1 = load("bands"), load("ident"), load("ones1")

            LF = bands[:, 0:128]      # fwd same-tile band (lhsT layout)
            UF = bands[:, 128:256]    # fwd prev-tile corner
            LR = bands[:, 256:384]    # rev same-tile band
            UR = bands[:, 384:512]    # rev next-tile corner

            # persistent tensors written on-device
            xhr_p = consts.tile([128, n2], f16, tag="xhr_p")
            xhr_no = consts.tile([128, n2], f16, tag="xhr_no")
            xhr_ni = consts.tile([128, n2], f16, tag="xhr_ni")
            outst = consts.tile([128, Tn * 64], f32, tag="outst")
            # X-part (rows 0:64) of the XHR stationaries never changes
            nc.vector.tensor_copy(xhr_p[0:64, :], xht[0:64, :])
            nc.vector.tensor_copy(xhr_no[0:64, :], xhno[0:64, :])
            nc.vector.tensor_copy(xhr_ni[0:64, :], xhni[0:64, :])

            sc_pool = ctx.enter_context(tc.tile_pool(name="sc", bufs=5))
            sq_pool = ctx.enter_context(tc.tile_pool(name="sq", bufs=5))
            sch_pool = ctx.enter_context(tc.tile_pool(name="sch", bufs=5))
            sqh_pool = ctx.enter_context(tc.tile_pool(name="sqh", bufs=5))
            hhr_pool = ctx.enter_context(tc.tile_pool(name="hhr", bufs=3))
            rt_pool = ctx.enter_context(tc.tile_pool(name="rt", bufs=3))
            zsb_pool = ctx.enter_context(tc.tile_pool(name="zsb", bufs=6))
            fin_pool = ctx.enter_context(tc.tile_pool(name="fin", bufs=4))

            # PSUM: 8 banks total. "l1" tag holds the z/r and h gate
            # accumulators (each lives ~3 pipeline steps -> 6 concurrent);
            # "y" tag holds all short-lived transients (feature outputs,
            # level-2 prop outputs, the R transpose).
            py = ctx.enter_context(tc.tile_pool(name="py", bufs=2, space="PSUM"))
            pl1 = ctx.enter_context(tc.tile_pool(name="pl1", bufs=6, space="PSUM"))

            sc = [None] * Tn
            sq = [None] * Tn
            sch = [None] * Tn
            sqh = [None] * Tn
            l1 = [None] * Tn
            l1h = [None] * Tn
            zsb = [None] * Tn

            def xs(tsr, j):
                return tsr[:, j * 128:(j + 1) * 128]

            mm = nc.tensor.matmul

            for it in range(Tn + 6):
                # ---- P1: z/r feature matmuls for tile j -------------------
                if it < Tn:
                    j = it
                    y = py.tile([128, 512], f32, tag="y")
                    mm(y[:, 0:256], xs(xhno, j), wzr[:, 128:384],
                       start=True, stop=False)
                    mm(y[:, 256:512], xs(xhni, j), wzr[:, 384:640],
                       start=False, stop=True)
                    t1 = pl1.tile([128, 128], f32, tag="l1")
                    mm(t1[:], xs(xht, j), wzr[:, 0:128],
                       start=True, stop=False)
                    if has_bias:
                        mm(t1[:], ones1[:, :], bzr[:, :],
                           start=False, stop=False)
                    l1[j] = t1
                    s = sc_pool.tile([128, 512], f16, tag="sc")
                    nc.vector.tensor_copy(s[:], y[:])
                    sc[j] = s

                # ---- P2: z/r level-1 band props for out-tile j ------------
                if 1 <= it <= Tn:
                    j = it - 1
                    t1 = l1[j]
                    # hh accumulators (group stays open until P3)
                    mm(t1[:], LF, sc[j][:, 0:128], start=False, stop=False)
                    mm(t1[:], LR, sc[j][:, 256:384], start=False, stop=False)
                    if j > 0:
                        mm(t1[:], UF, sc[j - 1][:, 0:128], start=False, stop=False)
                    if j < Tn - 1:
                        mm(t1[:], UR, sc[j + 1][:, 256:384], start=False, stop=False)
                    # q1 in a short-lived transient bank (own group)
                    q1 = py.tile([128, 256], f32, tag="y")
                    mms = [(q1[:, 0:128], LF, sc[j][:, 128:256]),
                           (q1[:, 128:256], LR, sc[j][:, 384:512])]
                    if j > 0:
                        mms.append((q1[:, 0:128], UF, sc[j - 1][:, 128:256]))
                    if j < Tn - 1:
                        mms.append((q1[:, 128:256], UR, sc[j + 1][:, 384:512]))
                    for i, (dst, st, mv) in enumerate(mms):
                        mm(dst, st, mv, start=(i == 0), stop=(i == len(mms) - 1))
                    q = sq_pool.tile([128, 256], f16, tag="sq")
                    nc.vector.tensor_scalar_mul(q[:, 0:128], q1[:, 0:128],
                                                nout[:, j:j + 1])
                    nc.vector.tensor_scalar_mul(q[:, 128:256], q1[:, 128:256],
                                                nin[:, j:j + 1])
                    sq[j] = q

                # ---- P3: z/r level-2 band props + gate finish -------------
                if 2 <= it <= Tn + 1:
                    j = it - 2
                    t1 = l1[j]
                    mms = [(LF, sq[j][:, 0:128])]
                    if j > 0:
                        mms.append((UF, sq[j - 1][:, 0:128]))
                    mms.append((LR, sq[j][:, 128:256]))
                    if j < Tn - 1:
                        mms.append((UR, sq[j + 1][:, 128:256]))
                    for i, (st, mv) in enumerate(mms):
                        mm(t1[:], st, mv, start=False, stop=(i == len(mms) - 1))
                    # Z gate: sigmoid straight from PSUM
                    z = zsb_pool.tile([128, 64], f32, tag="zsb")
                    nc.scalar.activation(z[:], t1[:, 0:64], SIG)
                    zsb[j] = z
                    # R gate: transpose to channel-major, then sigmoid
                    hr = hhr_pool.tile([128, 64], f16, tag="hhr")
                    nc.vector.tensor_copy(hr[:], t1[:, 64:128])
                    rtp = py.tile([128, 128], f16, tag="y")
                    nc.tensor.transpose(rtp[64:128, :], hr[:], ident[:])
                    rts = rt_pool.tile([128, 128], f16, tag="rt")
                    nc.scalar.activation(rts[64:128, :], rtp[64:128, :], SIG)
                    # XHR stationaries, H-part rows 64:128
                    nc.vector.tensor_mul(xs(xhr_p, j)[64:128, :],
                                         xs(xht, j)[64:128, :], rts[64:128, :])
                    nc.vector.tensor_mul(xs(xhr_no, j)[64:128, :],
                                         xs(xhno, j)[64:128, :], rts[64:128, :])
                    nc.vector.tensor_mul(xs(xhr_ni, j)[64:128, :],
                                         xs(xhni, j)[64:128, :], rts[64:128, :])

                # ---- P5: h-gate feature matmuls ---------------------------
                if 3 <= it <= Tn + 2:
                    j = it - 3
                    yh = py.tile([128, 256], f32, tag="y")
                    mm(yh[:, 0:128], xs(xhr_no, j), wh[:, 64:192],
                       start=True, stop=False)
                    mm(yh[:, 128:256], xs(xhr_ni, j), wh[:, 192:320],
                       start=False, stop=True)
                    th = pl1.tile([128, 128], f32, tag="l1")
                    mm(th[:, 0:64], xs(xhr_p, j), wh[:, 0:64],
                       start=True, stop=False)
                    if has_bias:
                        mm(th[:, 0:64], ones1[:, :], bh1[:, :],
                           start=False, stop=False)
                    l1h[j] = th
                    s = sch_pool.tile([128, 256], f16, tag="sch")
                    nc.vector.tensor_copy(s[:], yh[:])
                    sch[j] = s

                # ---- P6: h level-1 band props -----------------------------
                if 4 <= it <= Tn + 3:
                    j = it - 4
                    th = l1h[j]
                    hh = th[:, 0:64]
                    mm(hh, LF, sch[j][:, 0:64], start=False, stop=False)
                    mm(hh, LR, sch[j][:, 128:192], start=False, stop=False)
                    if j > 0:
                        mm(hh, UF, sch[j - 1][:, 0:64], start=False, stop=False)
                    if j < Tn - 1:
                        mm(hh, UR, sch[j + 1][:, 128:192], start=False, stop=False)
                    q1 = py.tile([128, 128], f32, tag="y")
                    mms = [(q1[:, 0:64], LF, sch[j][:, 64:128]),
                           (q1[:, 64:128], LR, sch[j][:, 192:256])]
                    if j > 0:
                        mms.append((q1[:, 0:64], UF, sch[j - 1][:, 64:128]))
                    if j < Tn - 1:
                        mms.append((q1[:, 64:128], UR, sch[j + 1][:, 192:256]))
                    for i, (dst, st, mv) in enumerate(mms):
                        mm(dst, st, mv, start=(i == 0), stop=(i == len(mms) - 1))
                    q = sqh_pool.tile([128, 128], f16, tag="sqh")
                    nc.vector.tensor_scalar_mul(q[:, 0:64], q1[:, 0:64],
                                                nout[:, j:j + 1])
                    nc.vector.tensor_scalar_mul(q[:, 64:128], q1[:, 64:128],
                                                nin[:, j:j + 1])
                    sqh[j] = q

                # ---- P7: h level-2 + H_tilde + final combine --------------
                if 5 <= it <= Tn + 4:
                    j = it - 5
                    th = l1h[j]
                    mms = [(LF, sqh[j][:, 0:64])]
                    if j > 0:
                        mms.append((UF, sqh[j - 1][:, 0:64]))
                    mms.append((LR, sqh[j][:, 64:128]))
                    if j < Tn - 1:
                        mms.append((UR, sqh[j + 1][:, 64:128]))
                    for i, (st, mv) in enumerate(mms):
                        mm(th[:, 0:64], st, mv, start=False,
                           stop=(i == len(mms) - 1))
                    ht = fin_pool.tile([128, 64], f32, tag="ht")
                    nc.scalar.activation(ht[:], th[:, 0:64], TANH)
                    # out = ht + z * (h - ht)
                    d = fin_pool.tile([128, 64], f32, tag="fd")
                    nc.vector.tensor_sub(d[:], hext[:, j * 64:(j + 1) * 64], ht[:])
                    e = fin_pool.tile([128, 64], f32, tag="fe")
                    nc.vector.tensor_mul(e[:], d[:], zsb[j][:])
                    nc.vector.tensor_add(outst[:, j * 64:(j + 1) * 64], e[:], ht[:])

            nc.sync.dma_start(d_out, outst[:])

    nc.compile()
    return nc


def _get_program(n_tiles=T, has_bias=False):
    key = (n_tiles, has_bias)
    if key not in _prog_cache:
        _prog_cache[key] = _build_program(n_tiles, has_bias)
    return _prog_cache[key]


# ----------------------------------------------------------------------------
# Host-side preparation
# ----------------------------------------------------------------------------
def _band_matrices():
    s = np.arange(128)[:, None]
    d = np.arange(128)[None, :]
    bf_l = ((d - s >= 1) & (d - s <= DEG)).astype(np.float16)
    bf_u = (s - d >= 128 - DEG).astype(np.float16)
    br_l = ((s - d >= 1) & (s - d <= DEG)).astype(np.float16)
    br_u = (d - s >= 128 - DEG).astype(np.float16)
    return np.concatenate([bf_l, bf_u, br_l, br_u], axis=1)  # [128, 512]


def _pack_weights_zr(Wz, Wr):
    def parts(W):
        w0 = W[0, 0] + W[1, 0] - W[0, 2] - W[1, 2]
        return w0, W[0, 1], 2.0 * W[0, 2], W[1, 1], 2.0 * W[1, 2]
    w0z, f1z, f2z, r1z, r2z = parts(Wz)
    w0r, f1r, f2r, r1r, r2r = parts(Wr)
    return np.concatenate(
        [w0z, w0r, f1z, f1r, f2z, f2r, r1z, r1r, r2z, r2r], axis=1
    ).astype(np.float16)  # [128, 640]


def _pack_weights_h(Wh):
    w0 = Wh[0, 0] + Wh[1, 0] - Wh[0, 2] - Wh[1, 2]
    return np.concatenate(
        [w0, Wh[0, 1], 2.0 * Wh[0, 2], Wh[1, 1], 2.0 * Wh[1, 2]], axis=1
    ).astype(np.float16)  # [128, 320]


def _is_banded_graph(row, col):
    if row.shape != (N * DEG,):
        return False
    r_exp = np.repeat(np.arange(N, dtype=np.int64), DEG)
    if not np.array_equal(row, r_exp):
        return False
    c_exp = (r_exp + np.tile(np.arange(1, DEG + 1, dtype=np.int64), N)) % N
    return np.array_equal(col, c_exp)


def _numpy_fallback(X, H, edge_weight, Wz, bz, Wr, br, Wh, bh, row, col):
    """Exact reference math on the host (only used if the graph deviates)."""
    deg_out = np.bincount(row, weights=edge_weight, minlength=N).astype(np.float32)
    deg_in = np.bincount(col, weights=edge_weight, minlength=N).astype(np.float32)
    norm_out = (1.0 / (deg_out + EPS))[row].astype(np.float32)
    norm_in = (1.0 / (deg_in + EPS))[col].astype(np.float32)

    def prop(x, src, dst, nrm):
        msg = x[:, src, :] * nrm[None, :, None]
        out = np.zeros_like(x)
        np.add.at(out, (slice(None), dst), msg)
        return out

    def dconv(Xc, W, b):
        Hh = Xc @ (W[0, 0] + W[1, 0])
        t1o = prop(Xc, row, col, norm_out)
        t1i = prop(Xc, col, row, norm_in)
        Hh = Hh + t1o @ W[0, 1] + t1i @ W[1, 1]
        for k in range(2, K):
            t1o = 2.0 * prop(t1o, row, col, norm_out) - Xc
            t1i = 2.0 * prop(t1i, col, row, norm_in) - Xc
            Hh = Hh + t1o @ W[0, k] + t1i @ W[1, k]
        return Hh + b

    XH = np.concatenate([X, H], axis=-1)
    Z = 1.0 / (1.0 + np.exp(-dconv(XH, Wz, bz)))
    R = 1.0 / (1.0 + np.exp(-dconv(XH, Wr, br)))
    XHR = np.concatenate([X, H * R], axis=-1)
    Ht = np.tanh(dconv(XHR, Wh, bh))
    return (Z * H + (1.0 - Z) * Ht).astype(np.float32)


def make_in_maps(X, H, edge_weight, Wz, bz, Wr, br, Wh, bh, row, col):
    """Build the 8 per-core input dicts (host sharding + preprocessing)."""
    deg_out = np.bincount(row, weights=edge_weight, minlength=N).astype(np.float32)
    deg_in = np.bincount(col, weights=edge_weight, minlength=N).astype(np.float32)
    n_out = (1.0 / (deg_out + EPS)).astype(np.float32)
    n_in = (1.0 / (deg_in + EPS)).astype(np.float32)

    shared = {
        "wzr": _pack_weights_zr(Wz, Wr),
        "wh": _pack_weights_h(Wh),
        "bzr": np.concatenate([bz, br])[None, :].astype(np.float16),
        "bh1": bh[None, :].astype(np.float16),
        "bands": _band_matrices(),
        "ident": np.eye(128, dtype=np.float16),
        "ones1": np.ones((1, 128), dtype=np.float16),
    }

    in_maps = []
    for core in range(NCORES):
        b, half = core // 2, core % 2
        g0 = half * HALF - LHALO
        idx = (g0 + np.arange(N2)) % N
        ext = np.concatenate([X[b], H[b]], axis=1)[idx]          # [N2, 128] f32
        no_e = n_out[idx]
        ni_e = n_in[idx]
        m = dict(shared)
        m["xht"] = np.ascontiguousarray(ext.T).astype(np.float16)
        m["xhno"] = np.ascontiguousarray((ext * no_e[:, None]).T).astype(np.float16)
        m["xhni"] = np.ascontiguousarray((ext * ni_e[:, None]).T).astype(np.float16)
        m["hext"] = np.ascontiguousarray(
            H[b][idx].reshape(T, 128, 64).transpose(1, 0, 2).reshape(128, T * 64)
        ).astype(np.float32)
        m["nout"] = np.ascontiguousarray(no_e.reshape(T, 128).T).astype(np.float32)
        m["nin"] = np.ascontiguousarray(ni_e.reshape(T, 128).T).astype(np.float32)
        in_maps.append(m)
    return in_maps


def unshard_outputs(results):
    out = np.empty((B, N, OUT), dtype=np.float32)
    for core in range(NCORES):
        b, half = core // 2, core % 2
        res = results[core]["out"]                                # [128, T*64]
        ext = res.reshape(128, T, 64).transpose(1, 0, 2).reshape(N2, 64)
        out[b, half * HALF:(half + 1) * HALF] = ext[LHALO:LHALO + HALF]
    return out


def kernel(X, H, edge_weight, Wz, bz, Wr, br, Wh, bh, edge_index):
    X = np.asarray(X, dtype=np.float32)
    H = np.asarray(H, dtype=np.float32)
    edge_weight = np.asarray(edge_weight, dtype=np.float32)
    Wz = np.asarray(Wz, dtype=np.float32)
    Wr = np.asarray(Wr, dtype=np.float32)
    Wh = np.asarray(Wh, dtype=np.float32)
    bz = np.asarray(bz, dtype=np.float32)
    br = np.asarray(br, dtype=np.float32)
    bh = np.asarray(bh, dtype=np.float32)
    ei = np.asarray(edge_index)
    row = ei[0].astype(np.int64)
    col = ei[1].astype(np.int64)

    if not _is_banded_graph(row, col):
        return _numpy_fallback(X, H, edge_weight, Wz, bz, Wr, br, Wh, bh,
                               row, col)

    from concourse import bass_utils

    has_bias = bool(np.any(bz) or np.any(br) or np.any(bh))
    nc = _get_program(T, has_bias)
    in_maps = make_in_maps(X, H, edge_weight, Wz, bz, Wr, br, Wh, bh, row, col)
    res = bass_utils.run_bass_kernel_spmd(nc, in_maps, list(range(NCORES)))
    return unshard_outputs(res.results)


# revision 9
# speedup vs baseline: 1.0302x; 1.0058x over previous
"""Trainium2 Bass kernel for nn_DCRNN_Layer (gnn_message_passing).

Strategy
--------
The reference graph is deterministic: node i has out-edges to (i+1..i+16) mod N.
Therefore each Chebyshev diffusion step is a banded circulant operator:

    prop_fwd(x)[d] = sum_{j=1..16} (n_out * x)[d-j]
    prop_rev(x)[d] = sum_{j=1..16} (n_in  * x)[d+j]

with n_out/n_in the per-node inverse (weighted) degrees.  Band application is
expressed as dense 128x128 {0,1} band-block matmuls on the TensorEngine (two
constant blocks per node-tile: same-tile band + neighbor-tile corner), and the
per-node scaling commutes with the feature matmuls so it is folded into
host-prescaled operands.

The gate computation is refactored "feature first" (props act on 64-wide gate
activations instead of the 128-wide state):

    Hh = Xc@W0' + Bf(Yf1) + Br(Yr1) + Bf(no*Bf(Yf2)) + Br(ni*Br(Yr2)) + bias
    W0' = W00+W10-W02-W12,  Yf1=(no*Xc)@W01, Yf2=(no*Xc)@(2*W02), ...

Sharding: 8 cores = batch(4) x node-halves(2).  Each core gets a 5248-node
extended shard (64-node halo each side + padding, circularly wrapped on the
host) so no inter-core communication is needed.  All matmul operands are fp16
(PSUM accumulation fp32): end-to-end absmax error vs the fp32 reference is
~2e-3 of scale (validated by host emulation).

kernel() is self-contained: host preprocessing (numpy) -> 8-core SPMD Bass
kernel -> host unshard.  If the edge structure ever deviates from the banded
pattern, a numpy fallback reproduces the reference exactly.
"""

import numpy as np
from contextlib import ExitStack

# Problem constants (hardcoded per contract).
B, N, DEG, IN, OUT, K = 4, 10000, 16, 64, 64, 3
C = IN + OUT          # 128
EPS = 1e-8
NCORES = 8
HALF = N // 2         # 5000 nodes per node-shard
T = 41                # node tiles per extended shard
N2 = T * 128          # 5248 extended shard nodes
LHALO = 64            # left halo (ext index of output node 0)

_prog_cache = {}


# ----------------------------------------------------------------------------
# Device program
# ----------------------------------------------------------------------------
def _build_program(n_tiles=T, has_bias=False):
    import concourse.bass as bass
    import concourse.bacc as bacc
    import concourse.tile as tile
    from concourse import mybir

    f16 = mybir.dt.float16
    f32 = mybir.dt.float32
    Tn = n_tiles
    n2 = Tn * 128

    nc = bacc.Bacc(
        "TRN2",
        target_bir_lowering=False,
        debug=False,
        enable_asserts=False,
        num_devices=NCORES,
    )

    din = {}
    for name, shape, dt in [
        ("xht", [128, n2], f16),
        ("xhno", [128, n2], f16),
        ("xhni", [128, n2], f16),
        ("hext", [128, Tn * 64], f32),
        ("nout", [128, Tn], f32),
        ("nin", [128, Tn], f32),
        ("wzr", [128, 640], f16),
        ("wh", [128, 320], f16),
        ("bzr", [1, 128], f16),
        ("bh1", [1, 64], f16),
        ("bands", [128, 512], f16),
        ("ident", [128, 128], f16),
        ("ones1", [1, 128], f16),
    ]:
        din[name] = nc.dram_tensor(name, shape, dt, kind="ExternalInput").ap()
    d_out = nc.dram_tensor("out", [128, Tn * 64], f32, kind="ExternalOutput").ap()

    SIG = mybir.ActivationFunctionType.Sigmoid
    TANH = mybir.ActivationFunctionType.Tanh

    with tile.TileContext(nc) as tc:
        with ExitStack() as ctx:
            consts = ctx.enter_context(tc.tile_pool(name="consts", bufs=1))

            def load(name):
                t = consts.tile(din[name].shape, din[name].dtype, tag=name)
                nc.sync.dma_start(t[:], din[name])
                return t

            xht, xhno, xhni = load("xht"), load("xhno"), load("xhni")
            hext, nout, nin = load("hext"), load("nout"), load("nin")
            wzr, wh, bzr, bh1 = load("wzr"), load("wh"), load("bzr"), load("bh1")
            bands, ident, ones1 = load("bands"), load("ident"), load("ones1")

            LF = bands[:, 0:128]      # fwd same-tile band (lhsT layout)
            UF = bands[:, 128:256]    # fwd prev-tile corner
            LR = bands[:, 256:384]    # rev same-tile band
            UR = bands[:, 384:512]    # rev next-tile corner

            # persistent tensors written on-device
            xhr_p = consts.tile([128, n2], f16, tag="xhr_p")
            xhr_no = consts.tile([128, n2], f16, tag="xhr_no")
            xhr_ni = consts.tile([128, n2], f16, tag="xhr_ni")
            outst = consts.tile([128, Tn * 64], f32, tag="outst")
            # X-part (rows 0:64) of the XHR stationaries never changes
            nc.vector.tensor_copy(xhr_p[0:64, :], xht[0:64, :])
            nc.vector.tensor_copy(xhr_no[0:64, :], xhno[0:64, :])
            nc.vector.tensor_copy(xhr_ni[0:64, :], xhni[0:64, :])

            sc_pool = ctx.enter_context(tc.tile_pool(name="sc", bufs=5))
            sq_pool = ctx.enter_context(tc.tile_pool(name="sq", bufs=5))
            sch_pool = ctx.enter_context(tc.tile_pool(name="sch", bufs=5))
            sqh_pool = ctx.enter_context(tc.tile_pool(name="sqh", bufs=5))
            hhr_pool = ctx.enter_context(tc.tile_pool(name="hhr", bufs=3))
            rt_pool = ctx.enter_context(tc.tile_pool(name="rt", bufs=3))
            zsb_pool = ctx.enter_context(tc.tile_pool(name="zsb", bufs=6))
            fin_pool = ctx.enter_context(tc.tile_pool(name="fin", bufs=4))

            # PSUM: 8 banks total. "l1" tag holds the z/r and h gate
            # accumulators (each lives ~3 pipeline steps -> 6 concurrent);
            # "y" tag holds all short-lived transients (feature outputs,
            # level-2 prop outputs, the R transpose).
            py = ctx.enter_context(tc.tile_pool(name="py", bufs=4, space="PSUM"))
            pl1 = ctx.enter_context(tc.tile_pool(name="pl1", bufs=4, space="PSUM"))

            sc = [None] * Tn
            sq = [None] * Tn
            sch = [None] * Tn
            sqh = [None] * Tn
            l1 = [None] * Tn
            l1h = [None] * Tn
            zsb = [None] * Tn

            def xs(tsr, j):
                return tsr[:, j * 128:(j + 1) * 128]

            mm = nc.tensor.matmul

            for it in range(Tn + 6):
                # ---- P1: z/r feature matmuls for tile j -------------------
                if it < Tn:
                    j = it
                    y = py.tile([128, 512], f32, tag="y")
                    mm(y[:, 0:256], xs(xhno, j), wzr[:, 128:384],
                       start=True, stop=False)
                    mm(y[:, 256:512], xs(xhni, j), wzr[:, 384:640],
                       start=False, stop=True)
                    s = sc_pool.tile([128, 512], f16, tag="sc")
                    nc.vector.tensor_copy(s[:], y[:])
                    sc[j] = s

                # ---- P2: z/r level-1 band props for out-tile j ------------
                if 1 <= it <= Tn:
                    j = it - 1
                    t1 = pl1.tile([128, 128], f32, tag="l1")
                    l1[j] = t1
                    mm(t1[:], xs(xht, j), wzr[:, 0:128],
                       start=True, stop=False)
                    if has_bias:
                        mm(t1[:], ones1[:, :], bzr[:, :],
                           start=False, stop=False)
                    # hh accumulators (group stays open until P3)
                    mm(t1[:], LF, sc[j][:, 0:128], start=False, stop=False)
                    mm(t1[:], LR, sc[j][:, 256:384], start=False, stop=False)
                    if j > 0:
                        mm(t1[:], UF, sc[j - 1][:, 0:128], start=False, stop=False)
                    if j < Tn - 1:
                        mm(t1[:], UR, sc[j + 1][:, 256:384], start=False, stop=False)
                    # q1 in a short-lived transient bank (own group)
                    q1 = py.tile([128, 256], f32, tag="y")
                    mms = [(q1[:, 0:128], LF, sc[j][:, 128:256]),
                           (q1[:, 128:256], LR, sc[j][:, 384:512])]
                    if j > 0:
                        mms.append((q1[:, 0:128], UF, sc[j - 1][:, 128:256]))
                    if j < Tn - 1:
                        mms.append((q1[:, 128:256], UR, sc[j + 1][:, 384:512]))
                    for i, (dst, st, mv) in enumerate(mms):
                        mm(dst, st, mv, start=(i == 0), stop=(i == len(mms) - 1))
                    q = sq_pool.tile([128, 256], f16, tag="sq")
                    nc.vector.tensor_scalar_mul(q[:, 0:128], q1[:, 0:128],
                                                nout[:, j:j + 1])
                    nc.vector.tensor_scalar_mul(q[:, 128:256], q1[:, 128:256],
                                                nin[:, j:j + 1])
                    sq[j] = q

                # ---- P3: z/r level-2 band props + gate finish -------------
                if 2 <= it <= Tn + 1:
                    j = it - 2
                    t1 = l1[j]
                    mms = [(LF, sq[j][:, 0:128])]
                    if j > 0:
                        mms.append((UF, sq[j - 1][:, 0:128]))
                    mms.append((LR, sq[j][:, 128:256]))
                    if j < Tn - 1:
                        mms.append((UR, sq[j + 1][:, 128:256]))
                    for i, (st, mv) in enumerate(mms):
                        mm(t1[:], st, mv, start=False, stop=(i == len(mms) - 1))
                    # Z gate: sigmoid straight from PSUM
                    z = zsb_pool.tile([128, 64], f32, tag="zsb")
                    nc.scalar.activation(z[:], t1[:, 0:64], SIG)
                    zsb[j] = z
                    # R gate: transpose to channel-major, then sigmoid
                    hr = hhr_pool.tile([128, 64], f16, tag="hhr")
                    nc.vector.tensor_copy(hr[:], t1[:, 64:128])
                    rtp = py.tile([128, 128], f16, tag="y")
                    nc.tensor.transpose(rtp[64:128, :], hr[:], ident[:])
                    rts = rt_pool.tile([128, 128], f16, tag="rt")
                    nc.scalar.activation(rts[64:128, :], rtp[64:128, :], SIG)
                    # XHR stationaries, H-part rows 64:128
                    nc.vector.tensor_mul(xs(xhr_p, j)[64:128, :],
                                         xs(xht, j)[64:128, :], rts[64:128, :])
                    nc.vector.tensor_mul(xs(xhr_no, j)[64:128, :],
                                         xs(xhno, j)[64:128, :], rts[64:128, :])
                    nc.vector.tensor_mul(xs(xhr_ni, j)[64:128, :],
                                         xs(xhni, j)[64:128, :], rts[64:128, :])

                # ---- P5: h-gate feature matmuls ---------------------------
                if 3 <= it <= Tn + 2:
                    j = it - 3
                    yh = py.tile([128, 256], f32, tag="y")
                    mm(yh[:, 0:128], xs(xhr_no, j), wh[:, 64:192],
                       start=True, stop=False)
                    mm(yh[:, 128:256], xs(xhr_ni, j), wh[:, 192:320],
                       start=False, stop=True)
                    s = sch_pool.tile([128, 256], f16, tag="sch")
                    nc.vector.tensor_copy(s[:], yh[:])
                    sch[j] = s

                # ---- P6: h level-1 band props -----------------------------
                if 4 <= it <= Tn + 3:
                    j = it - 4
                    th = pl1.tile([128, 128], f32, tag="l1")
                    l1h[j] = th
                    hh = th[:, 0:64]
                    mm(hh, xs(xhr_p, j), wh[:, 0:64], start=True, stop=False)
                    if has_bias:
                        mm(hh, ones1[:, :], bh1[:, :], start=False, stop=False)
                    mm(hh, LF, sch[j][:, 0:64], start=False, stop=False)
                    mm(hh, LR, sch[j][:, 128:192], start=False, stop=False)
                    if j > 0:
                        mm(hh, UF, sch[j - 1][:, 0:64], start=False, stop=False)
                    if j < Tn - 1:
                        mm(hh, UR, sch[j + 1][:, 128:192], start=False, stop=False)
                    q1 = py.tile([128, 128], f32, tag="y")
                    mms = [(q1[:, 0:64], LF, sch[j][:, 64:128]),
                           (q1[:, 64:128], LR, sch[j][:, 192:256])]
                    if j > 0:
                        mms.append((q1[:, 0:64], UF, sch[j - 1][:, 64:128]))
                    if j < Tn - 1:
                        mms.append((q1[:, 64:128], UR, sch[j + 1][:, 192:256]))
                    for i, (dst, st, mv) in enumerate(mms):
                        mm(dst, st, mv, start=(i == 0), stop=(i == len(mms) - 1))
                    q = sqh_pool.tile([128, 128], f16, tag="sqh")
                    nc.vector.tensor_scalar_mul(q[:, 0:64], q1[:, 0:64],
                                                nout[:, j:j + 1])
                    nc.vector.tensor_scalar_mul(q[:, 64:128], q1[:, 64:128],
                                                nin[:, j:j + 1])
                    sqh[j] = q

                # ---- P7: h level-2 + H_tilde + final combine --------------
                if 5 <= it <= Tn + 4:
                    j = it - 5
                    th = l1h[j]
                    mms = [(LF, sqh[j][:, 0:64])]
                    if j > 0:
                        mms.append((UF, sqh[j - 1][:, 0:64]))
                    mms.append((LR, sqh[j][:, 64:128]))
                    if j < Tn - 1:
                        mms.append((UR, sqh[j + 1][:, 64:128]))
                    for i, (st, mv) in enumerate(mms):
                        mm(th[:, 0:64], st, mv, start=False,
                           stop=(i == len(mms) - 1))
                    ht = fin_pool.tile([128, 64], f32, tag="ht")
                    nc.scalar.activation(ht[:], th[:, 0:64], TANH)
                    # out = ht + z * (h - ht)
                    d = fin_pool.tile([128, 64], f32, tag="fd")
                    nc.vector.tensor_sub(d[:], hext[:, j * 64:(j + 1) * 64], ht[:])
                    e = fin_pool.tile([128, 64], f32, tag="fe")
                    nc.vector.tensor_mul(e[:], d[:], zsb[j][:])
                    nc.vector.tensor_add(outst[:, j * 64:(j + 1) * 64], e[:], ht[:])

            nc.sync.dma_start(d_out, outst[:])

    nc.compile()
    return nc


def _get_program(n_tiles=T, has_bias=False):
    key = (n_tiles, has_bias)
    if key not in _prog_cache:
        _prog_cache[key] = _build_program(n_tiles, has_bias)
    return _prog_cache[key]


# ----------------------------------------------------------------------------
# Host-side preparation
# ----------------------------------------------------------------------------
def _band_matrices():
    s = np.arange(128)[:, None]
    d = np.arange(128)[None, :]
    bf_l = ((d - s >= 1) & (d - s <= DEG)).astype(np.float16)
    bf_u = (s - d >= 128 - DEG).astype(np.float16)
    br_l = ((s - d >= 1) & (s - d <= DEG)).astype(np.float16)
    br_u = (d - s >= 128 - DEG).astype(np.float16)
    return np.concatenate([bf_l, bf_u, br_l, br_u], axis=1)  # [128, 512]


def _pack_weights_zr(Wz, Wr):
    def parts(W):
        w0 = W[0, 0] + W[1, 0] - W[0, 2] - W[1, 2]
        return w0, W[0, 1], 2.0 * W[0, 2], W[1, 1], 2.0 * W[1, 2]
    w0z, f1z, f2z, r1z, r2z = parts(Wz)
    w0r, f1r, f2r, r1r, r2r = parts(Wr)
    return np.concatenate(
        [w0z, w0r, f1z, f1r, f2z, f2r, r1z, r1r, r2z, r2r], axis=1
    ).astype(np.float16)  # [128, 640]


def _pack_weights_h(Wh):
    w0 = Wh[0, 0] + Wh[1, 0] - Wh[0, 2] - Wh[1, 2]
    return np.concatenate(
        [w0, Wh[0, 1], 2.0 * Wh[0, 2], Wh[1, 1], 2.0 * Wh[1, 2]], axis=1
    ).astype(np.float16)  # [128, 320]


def _is_banded_graph(row, col):
    if row.shape != (N * DEG,):
        return False
    r_exp = np.repeat(np.arange(N, dtype=np.int64), DEG)
    if not np.array_equal(row, r_exp):
        return False
    c_exp = (r_exp + np.tile(np.arange(1, DEG + 1, dtype=np.int64), N)) % N
    return np.array_equal(col, c_exp)


def _numpy_fallback(X, H, edge_weight, Wz, bz, Wr, br, Wh, bh, row, col):
    """Exact reference math on the host (only used if the graph deviates)."""
    deg_out = np.bincount(row, weights=edge_weight, minlength=N).astype(np.float32)
    deg_in = np.bincount(col, weights=edge_weight, minlength=N).astype(np.float32)
    norm_out = (1.0 / (deg_out + EPS))[row].astype(np.float32)
    norm_in = (1.0 / (deg_in + EPS))[col].astype(np.float32)

    def prop(x, src, dst, nrm):
        msg = x[:, src, :] * nrm[None, :, None]
        out = np.zeros_like(x)
        np.add.at(out, (slice(None), dst), msg)
        return out

    def dconv(Xc, W, b):
        Hh = Xc @ (W[0, 0] + W[1, 0])
        t1o = prop(Xc, row, col, norm_out)
        t1i = prop(Xc, col, row, norm_in)
        Hh = Hh + t1o @ W[0, 1] + t1i @ W[1, 1]
        for k in range(2, K):
            t1o = 2.0 * prop(t1o, row, col, norm_out) - Xc
            t1i = 2.0 * prop(t1i, col, row, norm_in) - Xc
            Hh = Hh + t1o @ W[0, k] + t1i @ W[1, k]
        return Hh + b

    XH = np.concatenate([X, H], axis=-1)
    Z = 1.0 / (1.0 + np.exp(-dconv(XH, Wz, bz)))
    R = 1.0 / (1.0 + np.exp(-dconv(XH, Wr, br)))
    XHR = np.concatenate([X, H * R], axis=-1)
    Ht = np.tanh(dconv(XHR, Wh, bh))
    return (Z * H + (1.0 - Z) * Ht).astype(np.float32)


def make_in_maps(X, H, edge_weight, Wz, bz, Wr, br, Wh, bh, row, col):
    """Build the 8 per-core input dicts (host sharding + preprocessing)."""
    deg_out = np.bincount(row, weights=edge_weight, minlength=N).astype(np.float32)
    deg_in = np.bincount(col, weights=edge_weight, minlength=N).astype(np.float32)
    n_out = (1.0 / (deg_out + EPS)).astype(np.float32)
    n_in = (1.0 / (deg_in + EPS)).astype(np.float32)

    shared = {
        "wzr": _pack_weights_zr(Wz, Wr),
        "wh": _pack_weights_h(Wh),
        "bzr": np.concatenate([bz, br])[None, :].astype(np.float16),
        "bh1": bh[None, :].astype(np.float16),
        "bands": _band_matrices(),
        "ident": np.eye(128, dtype=np.float16),
        "ones1": np.ones((1, 128), dtype=np.float16),
    }

    in_maps = []
    for core in range(NCORES):
        b, half = core // 2, core % 2
        g0 = half * HALF - LHALO
        idx = (g0 + np.arange(N2)) % N
        ext = np.concatenate([X[b], H[b]], axis=1)[idx]          # [N2, 128] f32
        no_e = n_out[idx]
        ni_e = n_in[idx]
        m = dict(shared)
        m["xht"] = np.ascontiguousarray(ext.T).astype(np.float16)
        m["xhno"] = np.ascontiguousarray((ext * no_e[:, None]).T).astype(np.float16)
        m["xhni"] = np.ascontiguousarray((ext * ni_e[:, None]).T).astype(np.float16)
        m["hext"] = np.ascontiguousarray(
            H[b][idx].reshape(T, 128, 64).transpose(1, 0, 2).reshape(128, T * 64)
        ).astype(np.float32)
        m["nout"] = np.ascontiguousarray(no_e.reshape(T, 128).T).astype(np.float32)
        m["nin"] = np.ascontiguousarray(ni_e.reshape(T, 128).T).astype(np.float32)
        in_maps.append(m)
    return in_maps


def unshard_outputs(results):
    out = np.empty((B, N, OUT), dtype=np.float32)
    for core in range(NCORES):
        b, half = core // 2, core % 2
        res = results[core]["out"]                                # [128, T*64]
        ext = res.reshape(128, T, 64).transpose(1, 0, 2).reshape(N2, 64)
        out[b, half * HALF:(half + 1) * HALF] = ext[LHALO:LHALO + HALF]
    return out


def kernel(X, H, edge_weight, Wz, bz, Wr, br, Wh, bh, edge_index):
    X = np.asarray(X, dtype=np.float32)
    H = np.asarray(H, dtype=np.float32)
    edge_weight = np.asarray(edge_weight, dtype=np.float32)
    Wz = np.asarray(Wz, dtype=np.float32)
    Wr = np.asarray(Wr, dtype=np.float32)
    Wh = np.asarray(Wh, dtype=np.float32)
    bz = np.asarray(bz, dtype=np.float32)
    br = np.asarray(br, dtype=np.float32)
    bh = np.asarray(bh, dtype=np.float32)
    ei = np.asarray(edge_index)
    row = ei[0].astype(np.int64)
    col = ei[1].astype(np.int64)

    if not _is_banded_graph(row, col):
        return _numpy_fallback(X, H, edge_weight, Wz, bz, Wr, br, Wh, bh,
                               row, col)

    from concourse import bass_utils

    has_bias = bool(np.any(bz) or np.any(br) or np.any(bh))
    nc = _get_program(T, has_bias)
    in_maps = make_in_maps(X, H, edge_weight, Wz, bz, Wr, br, Wh, bh, row, col)
    res = bass_utils.run_bass_kernel_spmd(nc, in_maps, list(range(NCORES)))
    return unshard_outputs(res.results)


# revision 10
# speedup vs baseline: 1.1732x; 1.1388x over previous
"""Trainium2 Bass kernel for nn_DCRNN_Layer (gnn_message_passing).

Strategy
--------
The reference graph is deterministic: node i has out-edges to (i+1..i+16) mod N.
Therefore each Chebyshev diffusion step is a banded circulant operator:

    prop_fwd(x)[d] = sum_{j=1..16} (n_out * x)[d-j]
    prop_rev(x)[d] = sum_{j=1..16} (n_in  * x)[d+j]

with n_out/n_in the per-node inverse (weighted) degrees.  Band application is
expressed as dense 128x128 {0,1} band-block matmuls on the TensorEngine (two
constant blocks per node-tile: same-tile band + neighbor-tile corner), and the
per-node scaling commutes with the feature matmuls so it is folded into
host-prescaled operands.

The gate computation is refactored "feature first" (props act on 64-wide gate
activations instead of the 128-wide state):

    Hh = Xc@W0' + Bf(Yf1) + Br(Yr1) + Bf(no*Bf(Yf2)) + Br(ni*Br(Yr2)) + bias
    W0' = W00+W10-W02-W12,  Yf1=(no*Xc)@W01, Yf2=(no*Xc)@(2*W02), ...

Sharding: 8 cores = batch(4) x node-halves(2).  Each core gets a 5248-node
extended shard (64-node halo each side + padding, circularly wrapped on the
host) so no inter-core communication is needed.  All matmul operands are fp16
(PSUM accumulation fp32): end-to-end absmax error vs the fp32 reference is
~2e-3 of scale (validated by host emulation).

kernel() is self-contained: host preprocessing (numpy) -> 8-core SPMD Bass
kernel -> host unshard.  If the edge structure ever deviates from the banded
pattern, a numpy fallback reproduces the reference exactly.
"""

import numpy as np
from contextlib import ExitStack

# Problem constants (hardcoded per contract).
B, N, DEG, IN, OUT, K = 4, 10000, 16, 64, 64, 3
C = IN + OUT          # 128
EPS = 1e-8
NCORES = 8
HALF = N // 2         # 5000 nodes per node-shard
T = 41                # node tiles per extended shard
N2 = T * 128          # 5248 extended shard nodes
LHALO = 64            # left halo (ext index of output node 0)

_prog_cache = {}


# ----------------------------------------------------------------------------
# Device program
# ----------------------------------------------------------------------------
def _build_program(n_tiles=T, has_bias=False):
    import concourse.bass as bass
    import concourse.bacc as bacc
    import concourse.tile as tile
    from concourse import mybir

    f16 = mybir.dt.float16
    f32 = mybir.dt.float32
    Tn = n_tiles
    n2 = Tn * 128

    nc = bacc.Bacc(
        "TRN2",
        target_bir_lowering=False,
        debug=False,
        enable_asserts=False,
        num_devices=NCORES,
    )

    din = {}
    for name, shape, dt in [
        ("xht", [128, n2], f16),
        ("xhno", [128, n2], f16),
        ("xhni", [128, n2], f16),
        ("hext", [128, Tn * 64], f32),
        ("nout", [128, Tn], f32),
        ("nin", [128, Tn], f32),
        ("wzr", [128, 640], f16),
        ("wh", [128, 320], f16),
        ("bzr", [1, 128], f16),
        ("bh1", [1, 64], f16),
        ("bands", [128, 512], f16),
        ("ident", [128, 128], f16),
        ("ones1", [1, 128], f16),
    ]:
        din[name] = nc.dram_tensor(name, shape, dt, kind="ExternalInput").ap()
    d_out = nc.dram_tensor("out", [128, Tn * 64], f32, kind="ExternalOutput").ap()

    SIG = mybir.ActivationFunctionType.Sigmoid
    TANH = mybir.ActivationFunctionType.Tanh

    with tile.TileContext(nc) as tc:
        with ExitStack() as ctx:
            consts = ctx.enter_context(tc.tile_pool(name="consts", bufs=1))

            def load(name):
                t = consts.tile(din[name].shape, din[name].dtype, tag=name)
                nc.sync.dma_start(t[:], din[name])
                return t

            xht, xhno, xhni = load("xht"), load("xhno"), load("xhni")
            hext, nout, nin = load("hext"), load("nout"), load("nin")
            wzr, wh, bzr, bh1 = load("wzr"), load("wh"), load("bzr"), load("bh1")
            bands, ident, ones1 = load("bands"), load("ident"), load("ones1")

            LF = bands[:, 0:128]      # fwd same-tile band (lhsT layout)
            UF = bands[:, 128:256]    # fwd prev-tile corner
            LR = bands[:, 256:384]    # rev same-tile band
            UR = bands[:, 384:512]    # rev next-tile corner

            # persistent tensors written on-device
            xhr_p = consts.tile([128, n2], f16, tag="xhr_p")
            xhr_no = consts.tile([128, n2], f16, tag="xhr_no")
            xhr_ni = consts.tile([128, n2], f16, tag="xhr_ni")
            outst = consts.tile([128, Tn * 64], f32, tag="outst")
            # X-part (rows 0:64) of the XHR stationaries never changes
            nc.vector.tensor_copy(xhr_p[0:64, :], xht[0:64, :])
            nc.vector.tensor_copy(xhr_no[0:64, :], xhno[0:64, :])
            nc.vector.tensor_copy(xhr_ni[0:64, :], xhni[0:64, :])

            sc_pool = ctx.enter_context(tc.tile_pool(name="sc", bufs=5))
            sq_pool = ctx.enter_context(tc.tile_pool(name="sq", bufs=5))
            sch_pool = ctx.enter_context(tc.tile_pool(name="sch", bufs=5))
            sqh_pool = ctx.enter_context(tc.tile_pool(name="sqh", bufs=5))
            hhr_pool = ctx.enter_context(tc.tile_pool(name="hhr", bufs=3))
            rt_pool = ctx.enter_context(tc.tile_pool(name="rt", bufs=3))
            zsb_pool = ctx.enter_context(tc.tile_pool(name="zsb", bufs=6))
            fin_pool = ctx.enter_context(tc.tile_pool(name="fin", bufs=4))

            # PSUM: 8 banks total. "l1" tag holds the z/r and h gate
            # accumulators (each lives ~3 pipeline steps -> 6 concurrent);
            # "y" tag holds all short-lived transients (feature outputs,
            # level-2 prop outputs, the R transpose).
            py = ctx.enter_context(tc.tile_pool(name="py", bufs=4, space="PSUM"))
            pl1 = ctx.enter_context(tc.tile_pool(name="pl1", bufs=4, space="PSUM"))

            sc = [None] * Tn
            sq = [None] * Tn
            sch = [None] * Tn
            sqh = [None] * Tn
            l1 = [None] * Tn
            l1h = [None] * Tn
            zsb = [None] * Tn

            def xs(tsr, j):
                return tsr[:, j * 128:(j + 1) * 128]

            mm = nc.tensor.matmul

            for it in range(Tn + 6):
                # ---- P1: z/r feature matmuls for tile j -------------------
                if it < Tn:
                    j = it
                    y = py.tile([128, 512], f32, tag="y")
                    mm(y[:, 0:256], xs(xhno, j), wzr[:, 128:384],
                       start=True, stop=False)
                    mm(y[:, 256:512], xs(xhni, j), wzr[:, 384:640],
                       start=False, stop=True)
                    s = sc_pool.tile([128, 512], f16, tag="sc")
                    nc.vector.tensor_copy(s[:], y[:])
                    sc[j] = s

                # ---- P2: z/r level-1 band props for out-tile j ------------
                if 1 <= it <= Tn:
                    j = it - 1
                    t1 = pl1.tile([128, 128], f32, tag="l1")
                    l1[j] = t1
                    mm(t1[:], xs(xht, j), wzr[:, 0:128],
                       start=True, stop=False)
                    if has_bias:
                        mm(t1[:], ones1[:, :], bzr[:, :],
                           start=False, stop=False)
                    # hh accumulators (group stays open until P3)
                    mm(t1[:], LF, sc[j][:, 0:128], start=False, stop=False)
                    mm(t1[:], LR, sc[j][:, 256:384], start=False, stop=False)
                    if j > 0:
                        mm(t1[:], UF, sc[j - 1][:, 0:128], start=False, stop=False)
                    if j < Tn - 1:
                        mm(t1[:], UR, sc[j + 1][:, 256:384], start=False, stop=False)
                    # q1 in a short-lived transient bank (own group)
                    q1 = py.tile([128, 256], f32, tag="y")
                    mms = [(q1[:, 0:128], LF, sc[j][:, 128:256]),
                           (q1[:, 128:256], LR, sc[j][:, 384:512])]
                    if j > 0:
                        mms.append((q1[:, 0:128], UF, sc[j - 1][:, 128:256]))
                    if j < Tn - 1:
                        mms.append((q1[:, 128:256], UR, sc[j + 1][:, 384:512]))
                    for i, (dst, st, mv) in enumerate(mms):
                        mm(dst, st, mv, start=(i == 0), stop=(i == len(mms) - 1))
                    q = sq_pool.tile([128, 256], f16, tag="sq")
                    nc.vector.tensor_scalar_mul(q[:, 0:128], q1[:, 0:128],
                                                nout[:, j:j + 1])
                    nc.vector.tensor_scalar_mul(q[:, 128:256], q1[:, 128:256],
                                                nin[:, j:j + 1])
                    sq[j] = q

                # ---- P3: z/r level-2 band props + gate finish -------------
                if 2 <= it <= Tn + 1:
                    j = it - 2
                    t1 = l1[j]
                    mms = [(LF, sq[j][:, 0:128])]
                    if j > 0:
                        mms.append((UF, sq[j - 1][:, 0:128]))
                    mms.append((LR, sq[j][:, 128:256]))
                    if j < Tn - 1:
                        mms.append((UR, sq[j + 1][:, 128:256]))
                    for i, (st, mv) in enumerate(mms):
                        mm(t1[:], st, mv, start=False, stop=(i == len(mms) - 1))
                    # Z gate: sigmoid straight from PSUM
                    z = zsb_pool.tile([128, 64], f32, tag="zsb")
                    nc.scalar.activation(z[:], t1[:, 0:64], SIG)
                    zsb[j] = z
                    # R gate: transpose to channel-major, then sigmoid
                    hr = hhr_pool.tile([128, 64], f16, tag="hhr")
                    nc.vector.tensor_copy(hr[:], t1[:, 64:128])
                    rtp = py.tile([128, 128], f16, tag="y")
                    nc.tensor.transpose(rtp[64:128, :], hr[:], ident[:])
                    rts = rt_pool.tile([128, 128], f16, tag="rt")
                    nc.scalar.activation(rts[64:128, :], rtp[64:128, :], SIG)
                    # XHR stationaries, H-part rows 64:128
                    nc.gpsimd.tensor_mul(xs(xhr_p, j)[64:128, :],
                                         xs(xht, j)[64:128, :], rts[64:128, :])
                    nc.gpsimd.tensor_mul(xs(xhr_no, j)[64:128, :],
                                         xs(xhno, j)[64:128, :], rts[64:128, :])
                    nc.gpsimd.tensor_mul(xs(xhr_ni, j)[64:128, :],
                                         xs(xhni, j)[64:128, :], rts[64:128, :])

                # ---- P5: h-gate feature matmuls ---------------------------
                if 3 <= it <= Tn + 2:
                    j = it - 3
                    yh = py.tile([128, 256], f32, tag="y")
                    mm(yh[:, 0:128], xs(xhr_no, j), wh[:, 64:192],
                       start=True, stop=False)
                    mm(yh[:, 128:256], xs(xhr_ni, j), wh[:, 192:320],
                       start=False, stop=True)
                    s = sch_pool.tile([128, 256], f16, tag="sch")
                    nc.vector.tensor_copy(s[:], yh[:])
                    sch[j] = s

                # ---- P6: h level-1 band props -----------------------------
                if 4 <= it <= Tn + 3:
                    j = it - 4
                    th = pl1.tile([128, 128], f32, tag="l1")
                    l1h[j] = th
                    hh = th[:, 0:64]
                    mm(hh, xs(xhr_p, j), wh[:, 0:64], start=True, stop=False)
                    if has_bias:
                        mm(hh, ones1[:, :], bh1[:, :], start=False, stop=False)
                    mm(hh, LF, sch[j][:, 0:64], start=False, stop=False)
                    mm(hh, LR, sch[j][:, 128:192], start=False, stop=False)
                    if j > 0:
                        mm(hh, UF, sch[j - 1][:, 0:64], start=False, stop=False)
                    if j < Tn - 1:
                        mm(hh, UR, sch[j + 1][:, 128:192], start=False, stop=False)
                    q1 = py.tile([128, 128], f32, tag="y")
                    mms = [(q1[:, 0:64], LF, sch[j][:, 64:128]),
                           (q1[:, 64:128], LR, sch[j][:, 192:256])]
                    if j > 0:
                        mms.append((q1[:, 0:64], UF, sch[j - 1][:, 64:128]))
                    if j < Tn - 1:
                        mms.append((q1[:, 64:128], UR, sch[j + 1][:, 192:256]))
                    for i, (dst, st, mv) in enumerate(mms):
                        mm(dst, st, mv, start=(i == 0), stop=(i == len(mms) - 1))
                    q = sqh_pool.tile([128, 128], f16, tag="sqh")
                    nc.vector.tensor_scalar_mul(q[:, 0:64], q1[:, 0:64],
                                                nout[:, j:j + 1])
                    nc.vector.tensor_scalar_mul(q[:, 64:128], q1[:, 64:128],
                                                nin[:, j:j + 1])
                    sqh[j] = q

                # ---- P7: h level-2 + H_tilde + final combine --------------
                if 5 <= it <= Tn + 4:
                    j = it - 5
                    th = l1h[j]
                    mms = [(LF, sqh[j][:, 0:64])]
                    if j > 0:
                        mms.append((UF, sqh[j - 1][:, 0:64]))
                    mms.append((LR, sqh[j][:, 64:128]))
                    if j < Tn - 1:
                        mms.append((UR, sqh[j + 1][:, 64:128]))
                    for i, (st, mv) in enumerate(mms):
                        mm(th[:, 0:64], st, mv, start=False,
                           stop=(i == len(mms) - 1))
                    ht = fin_pool.tile([128, 64], f32, tag="ht")
                    nc.scalar.activation(ht[:], th[:, 0:64], TANH)
                    # out = ht + z * (h - ht)
                    d = fin_pool.tile([128, 64], f32, tag="fd")
                    nc.gpsimd.tensor_sub(d[:], hext[:, j * 64:(j + 1) * 64], ht[:])
                    e = fin_pool.tile([128, 64], f32, tag="fe")
                    nc.gpsimd.tensor_mul(e[:], d[:], zsb[j][:])
                    nc.gpsimd.tensor_add(outst[:, j * 64:(j + 1) * 64], e[:], ht[:])

            nc.sync.dma_start(d_out, outst[:])

    nc.compile()
    return nc


def _get_program(n_tiles=T, has_bias=False):
    key = (n_tiles, has_bias)
    if key not in _prog_cache:
        _prog_cache[key] = _build_program(n_tiles, has_bias)
    return _prog_cache[key]


# ----------------------------------------------------------------------------
# Host-side preparation
# ----------------------------------------------------------------------------
def _band_matrices():
    s = np.arange(128)[:, None]
    d = np.arange(128)[None, :]
    bf_l = ((d - s >= 1) & (d - s <= DEG)).astype(np.float16)
    bf_u = (s - d >= 128 - DEG).astype(np.float16)
    br_l = ((s - d >= 1) & (s - d <= DEG)).astype(np.float16)
    br_u = (d - s >= 128 - DEG).astype(np.float16)
    return np.concatenate([bf_l, bf_u, br_l, br_u], axis=1)  # [128, 512]


def _pack_weights_zr(Wz, Wr):
    def parts(W):
        w0 = W[0, 0] + W[1, 0] - W[0, 2] - W[1, 2]
        return w0, W[0, 1], 2.0 * W[0, 2], W[1, 1], 2.0 * W[1, 2]
    w0z, f1z, f2z, r1z, r2z = parts(Wz)
    w0r, f1r, f2r, r1r, r2r = parts(Wr)
    return np.concatenate(
        [w0z, w0r, f1z, f1r, f2z, f2r, r1z, r1r, r2z, r2r], axis=1
    ).astype(np.float16)  # [128, 640]


def _pack_weights_h(Wh):
    w0 = Wh[0, 0] + Wh[1, 0] - Wh[0, 2] - Wh[1, 2]
    return np.concatenate(
        [w0, Wh[0, 1], 2.0 * Wh[0, 2], Wh[1, 1], 2.0 * Wh[1, 2]], axis=1
    ).astype(np.float16)  # [128, 320]


def _is_banded_graph(row, col):
    if row.shape != (N * DEG,):
        return False
    r_exp = np.repeat(np.arange(N, dtype=np.int64), DEG)
    if not np.array_equal(row, r_exp):
        return False
    c_exp = (r_exp + np.tile(np.arange(1, DEG + 1, dtype=np.int64), N)) % N
    return np.array_equal(col, c_exp)


def _numpy_fallback(X, H, edge_weight, Wz, bz, Wr, br, Wh, bh, row, col):
    """Exact reference math on the host (only used if the graph deviates)."""
    deg_out = np.bincount(row, weights=edge_weight, minlength=N).astype(np.float32)
    deg_in = np.bincount(col, weights=edge_weight, minlength=N).astype(np.float32)
    norm_out = (1.0 / (deg_out + EPS))[row].astype(np.float32)
    norm_in = (1.0 / (deg_in + EPS))[col].astype(np.float32)

    def prop(x, src, dst, nrm):
        msg = x[:, src, :] * nrm[None, :, None]
        out = np.zeros_like(x)
        np.add.at(out, (slice(None), dst), msg)
        return out

    def dconv(Xc, W, b):
        Hh = Xc @ (W[0, 0] + W[1, 0])
        t1o = prop(Xc, row, col, norm_out)
        t1i = prop(Xc, col, row, norm_in)
        Hh = Hh + t1o @ W[0, 1] + t1i @ W[1, 1]
        for k in range(2, K):
            t1o = 2.0 * prop(t1o, row, col, norm_out) - Xc
            t1i = 2.0 * prop(t1i, col, row, norm_in) - Xc
            Hh = Hh + t1o @ W[0, k] + t1i @ W[1, k]
        return Hh + b

    XH = np.concatenate([X, H], axis=-1)
    Z = 1.0 / (1.0 + np.exp(-dconv(XH, Wz, bz)))
    R = 1.0 / (1.0 + np.exp(-dconv(XH, Wr, br)))
    XHR = np.concatenate([X, H * R], axis=-1)
    Ht = np.tanh(dconv(XHR, Wh, bh))
    return (Z * H + (1.0 - Z) * Ht).astype(np.float32)


def make_in_maps(X, H, edge_weight, Wz, bz, Wr, br, Wh, bh, row, col):
    """Build the 8 per-core input dicts (host sharding + preprocessing)."""
    deg_out = np.bincount(row, weights=edge_weight, minlength=N).astype(np.float32)
    deg_in = np.bincount(col, weights=edge_weight, minlength=N).astype(np.float32)
    n_out = (1.0 / (deg_out + EPS)).astype(np.float32)
    n_in = (1.0 / (deg_in + EPS)).astype(np.float32)

    shared = {
        "wzr": _pack_weights_zr(Wz, Wr),
        "wh": _pack_weights_h(Wh),
        "bzr": np.concatenate([bz, br])[None, :].astype(np.float16),
        "bh1": bh[None, :].astype(np.float16),
        "bands": _band_matrices(),
        "ident": np.eye(128, dtype=np.float16),
        "ones1": np.ones((1, 128), dtype=np.float16),
    }

    in_maps = []
    for core in range(NCORES):
        b, half = core // 2, core % 2
        g0 = half * HALF - LHALO
        idx = (g0 + np.arange(N2)) % N
        ext = np.concatenate([X[b], H[b]], axis=1)[idx]          # [N2, 128] f32
        no_e = n_out[idx]
        ni_e = n_in[idx]
        m = dict(shared)
        m["xht"] = np.ascontiguousarray(ext.T).astype(np.float16)
        m["xhno"] = np.ascontiguousarray((ext * no_e[:, None]).T).astype(np.float16)
        m["xhni"] = np.ascontiguousarray((ext * ni_e[:, None]).T).astype(np.float16)
        m["hext"] = np.ascontiguousarray(
            H[b][idx].reshape(T, 128, 64).transpose(1, 0, 2).reshape(128, T * 64)
        ).astype(np.float32)
        m["nout"] = np.ascontiguousarray(no_e.reshape(T, 128).T).astype(np.float32)
        m["nin"] = np.ascontiguousarray(ni_e.reshape(T, 128).T).astype(np.float32)
        in_maps.append(m)
    return in_maps


def unshard_outputs(results):
    out = np.empty((B, N, OUT), dtype=np.float32)
    for core in range(NCORES):
        b, half = core // 2, core % 2
        res = results[core]["out"]                                # [128, T*64]
        ext = res.reshape(128, T, 64).transpose(1, 0, 2).reshape(N2, 64)
        out[b, half * HALF:(half + 1) * HALF] = ext[LHALO:LHALO + HALF]
    return out


def kernel(X, H, edge_weight, Wz, bz, Wr, br, Wh, bh, edge_index):
    X = np.asarray(X, dtype=np.float32)
    H = np.asarray(H, dtype=np.float32)
    edge_weight = np.asarray(edge_weight, dtype=np.float32)
    Wz = np.asarray(Wz, dtype=np.float32)
    Wr = np.asarray(Wr, dtype=np.float32)
    Wh = np.asarray(Wh, dtype=np.float32)
    bz = np.asarray(bz, dtype=np.float32)
    br = np.asarray(br, dtype=np.float32)
    bh = np.asarray(bh, dtype=np.float32)
    ei = np.asarray(edge_index)
    row = ei[0].astype(np.int64)
    col = ei[1].astype(np.int64)

    if not _is_banded_graph(row, col):
        return _numpy_fallback(X, H, edge_weight, Wz, bz, Wr, br, Wh, bh,
                               row, col)

    from concourse import bass_utils

    has_bias = bool(np.any(bz) or np.any(br) or np.any(bh))
    nc = _get_program(T, has_bias)
    in_maps = make_in_maps(X, H, edge_weight, Wz, bz, Wr, br, Wh, bh, row, col)
    res = bass_utils.run_bass_kernel_spmd(nc, in_maps, list(range(NCORES)))
    return unshard_outputs(res.results)
